# Initial kernel scaffold
#
"""Trainium2 Bass kernel for nn_CascadedGruCell.

Reference computation (per batch row b, F=512, V=28):
    xm   = x @ K + b0;  hm = h @ R + b1          (GRU, reset_after)
    z    = sigmoid(xm_z + hm_z)
    r    = sigmoid(xm_r + hm_r)
    hcand= tanh(xm_h + r * hm_h)
    gru  = z*h + (1-z)*hcand
    WoY[b,v] = (emb @ Wo)[idx[b,v]]              (28-entry table gather)
    pred = softmax(WoY + h @ Uo + x @ Co + Bo)

Strategy: pure data parallel over 8 cores (8192 rows each). Host does
zero-FLOP layout prep (transposes / concatenation / padding); device does
all arithmetic. Per core:
  - PE: out[m,n] = sum_f W[f,m] * xT[f,n]  (weights stationary, batch on
    the moving dim, N=512) accumulating x- and h-side into PSUM [112,512],
    then PE transpose-back of 128-batch blocks into row-per-partition
    layout for elementwise work.
  - sigmoid via tanh (0.5+0.5*tanh(x/2)) so ACT keeps one table set
    {tanh, exp} loaded.
  - table gather: 28x (is_equal mask * (T[k]+1) at bf16, max-accumulate).
"""

import sys

for _p in ("/opt/trn_rl_repo", "/root/.axon_site/_ro/trn_rl_repo"):
    if _p not in sys.path:
        sys.path.insert(0, _p)

import ml_dtypes
import numpy as np

import concourse.bass as bass
import concourse.mybir as mybir
from concourse.tile import TileContext

B, F, V = 65536, 512, 28
NCORES = 8
BC = B // NCORES            # 8192 rows per core
MACRO = 512                 # batch rows per matmul macro-tile
NMACRO = BC // MACRO        # 16
SUBS = MACRO // 128         # 4 transpose sub-blocks per macro
FLATW = BC * V // 128       # 1792 free elems of the [128, *] flat layout
ROWG = BC // 128            # 64 row-groups of 28 in the flat layout

F32 = mybir.dt.float32
BF16 = mybir.dt.bfloat16
Alu = mybir.AluOpType
Act = mybir.ActivationFunctionType


def _patch_tail_drain():
    """The walrus build in this container rejects >1-2 sync waits on one
    CTRL instruction; TileContext's tail drain attaches one wait per live
    sem lane. Split them across single-wait nops."""
    from concourse.tile import TileContext as TC
    from bass_rust import ScopedClock, VectorClock

    if getattr(TC, "_drain_split_patched", False):
        return

    def _drain_and_barrier(self, tick_clock, wait_clock):
        gc = tick_clock.global_clock
        ticks = list(gc)
        n = len(ticks)
        seen = [0] * n
        for p in [i for i, t in enumerate(ticks) if t > 0]:
            vec = list(seen)
            vec[p] = ticks[p]
            nop = self.nc.sync.nop(nofuse=True, hint="tail_drain_split")
            wait_clock.add_sem_waits(
                nop.ins,
                ScopedClock({None: VectorClock(vec)}),
                ScopedClock({None: VectorClock(seen)}),
            )
            seen[p] = ticks[p]
        drain_inst = self.nc.sync.drain()
        wait_clock.add_sem_waits(
            drain_inst.ins,
            ScopedClock({None: gc}),
            ScopedClock({None: VectorClock(seen)}),
        )
        self.nc.all_engine_barrier()
        assert self.sems is not None
        popped = self.nc._tile_sem_poison_stack.pop()
        assert popped is self._sem_poison
        self.nc.clear_and_free_semaphores(list(self.sems.allocated().values()))
        self.nc.all_engine_barrier()

    TC._drain_and_barrier = _drain_and_barrier
    TC._drain_split_patched = True


def build_kernel():
    _patch_tail_drain()
    nc = bass.Bass()

    xT = nc.dram_tensor("xT", [F, BC], F32, kind="ExternalInput")
    hT = nc.dram_tensor("hT", [30, BC], F32, kind="ExternalInput")
    hflat = nc.dram_tensor("hflat", [128, FLATW], F32, kind="ExternalInput")
    idxbf = nc.dram_tensor("idxbf", [128, FLATW], BF16, kind="ExternalInput")
    WxA = nc.dram_tensor("WxA", [F, 112], F32, kind="ExternalInput")
    WhA = nc.dram_tensor("WhA", [30, 112], F32, kind="ExternalInput")
    WhB = nc.dram_tensor("WhB", [30, V], F32, kind="ExternalInput")
    embT = nc.dram_tensor("embT", [V, V], F32, kind="ExternalInput")
    Wo = nc.dram_tensor("Wo", [V, 1], F32, kind="ExternalInput")
    eye = nc.dram_tensor("eye", [112, 112], F32, kind="ExternalInput")

    pred_o = nc.dram_tensor("pred", [128, FLATW], F32, kind="ExternalOutput")
    gru_o = nc.dram_tensor("gru", [128, FLATW], F32, kind="ExternalOutput")

    with TileContext(nc) as tc:
        with (
            tc.tile_pool(name="const", bufs=1) as cpool,
            tc.tile_pool(name="flat", bufs=1) as fpool,
            tc.tile_pool(name="xtiles", bufs=3) as xpool,
            tc.tile_pool(name="work", bufs=3) as wpool,
            tc.tile_pool(name="psum", bufs=2, space="PSUM") as ppool,
            tc.tile_pool(name="psum1", bufs=1, space="PSUM") as ppool1,
        ):
            # ---- constants into SBUF ----
            wx_sb = cpool.tile([128, 4 * 112], F32, tag="wx")
            for g in range(4):
                nc.sync.dma_start(
                    wx_sb[:, g * 112:(g + 1) * 112],
                    WxA[g * 128:(g + 1) * 128, :],
                )
            wha_sb = cpool.tile([30, 112], F32, tag="wha")
            nc.sync.dma_start(wha_sb[:], WhA[:])
            whb_sb = cpool.tile([30, V], F32, tag="whb")
            nc.sync.dma_start(whb_sb[:], WhB[:])
            embT_sb = cpool.tile([V, V], F32, tag="embT")
            nc.sync.dma_start(embT_sb[:], embT[:])
            wo_sb = cpool.tile([V, 1], F32, tag="wo")
            nc.sync.dma_start(wo_sb[:], Wo[:])
            eye_sb = cpool.tile([112, 112], F32, tag="eye")
            nc.sync.dma_start(eye_sb[:], eye[:])

            hflat_sb = fpool.tile([128, FLATW], F32, tag="hflat")
            nc.sync.dma_start(hflat_sb[:], hflat[:])
            idx_sb = fpool.tile([128, FLATW], BF16, tag="idx")
            nc.sync.dma_start(idx_sb[:], idxbf[:])

            gru_sb = fpool.tile([128, FLATW], F32, tag="gru_out")
            pred_sb = fpool.tile([128, FLATW], F32, tag="pred_out")

            # ---- table = emb @ Wo, broadcast to all partitions, +1 ----
            ones_sb = cpool.tile([1, 128], F32, tag="ones")
            nc.vector.memset(ones_sb[:], 1.0)
            ps_t = ppool1.tile([1, V], F32, tag="ps_table")
            nc.tensor.matmul(ps_t[:], wo_sb[:], embT_sb[:], start=True, stop=True)
            tbl1 = cpool.tile([1, V], F32, tag="tbl1")
            nc.scalar.copy(tbl1[:], ps_t[:])
            ps_b = ppool1.tile([128, V], F32, tag="ps_tblb")
            nc.tensor.matmul(ps_b[:], ones_sb[:], tbl1[:], start=True, stop=True)
            tblB = cpool.tile([128, V], F32, tag="tblB")
            # T' = table + 1 (>0 so max-accumulation over a zero init works)
            nc.vector.tensor_scalar(tblB[:], ps_b[:], 1.0, None, Alu.add)

            # ---- WoY gather: woy' = max_k (idx==k) * T'[k]  (bf16) ----
            woy = fpool.tile([128, FLATW], BF16, tag="woy")
            nc.vector.memset(woy[:], 0.0)
            gtmp = wpool.tile([128, FLATW], BF16, tag="gtmp")
            for k in range(V):
                gtmp = wpool.tile([128, FLATW], BF16, tag="gtmp")
                nc.vector.tensor_scalar(
                    gtmp[:], idx_sb[:], float(k), tblB[:, k:k + 1],
                    Alu.is_equal, Alu.mult,
                )
                nc.vector.tensor_tensor(woy[:], woy[:], gtmp[:], Alu.max)

            # ---- main loop over macro-tiles ----
            for m in range(NMACRO):
                n0 = m * MACRO
                xt = [xpool.tile([128, MACRO], F32, tag=f"xt{g}") for g in range(4)]
                for g in range(4):
                    nc.sync.dma_start(
                        xt[g][:], xT[g * 128:(g + 1) * 128, n0:n0 + MACRO]
                    )
                ht = xpool.tile([30, MACRO], F32, tag="ht")
                nc.sync.dma_start(ht[:], hT[:, n0:n0 + MACRO])

                psX = ppool.tile([112, MACRO], F32, tag="psX")
                psH = ppool.tile([V, MACRO], F32, tag="psH")
                for g in range(4):
                    nc.tensor.matmul(
                        psX[:], wx_sb[:, g * 112:(g + 1) * 112], xt[g][:],
                        start=(g == 0), stop=False,
                    )
                nc.tensor.matmul(psX[:], wha_sb[:], ht[:], start=False, stop=True)
                nc.tensor.matmul(psH[:], whb_sb[:], ht[:], start=True, stop=True)

                # PSUM -> SBUF (one wide copy each), then PE transpose-back
                sbX = wpool.tile([112, MACRO], F32, tag="sbX")
                nc.scalar.copy(sbX[:], psX[:])
                sbH = wpool.tile([V, MACRO], F32, tag="sbH")
                nc.scalar.copy(sbH[:], psH[:])

                ptA = ppool.tile([128, SUBS * 112], F32, tag="ptA")
                ptB = ppool.tile([128, SUBS * V], F32, tag="ptB")
                for s in range(SUBS):
                    nc.tensor.transpose(
                        ptA[:, s * 112:(s + 1) * 112],
                        sbX[:, s * 128:(s + 1) * 128],
                        eye_sb[:],
                    )
                    nc.tensor.transpose(
                        ptB[:, s * V:(s + 1) * V],
                        sbH[:, s * 128:(s + 1) * 128],
                        eye_sb[0:V, 0:V],
                    )

                # row-per-partition views: ptA blocks [s] = [zr(56) xh(28) rest(28)]
                A = ptA[:].rearrange("p (s c) -> p s c", c=112)
                Bv = ptB[:].rearrange("p (s c) -> p s c", c=V)
                fsl = slice(SUBS * V * m, SUBS * V * (m + 1))
                hsl = hflat_sb[:, fsl].rearrange("p (s c) -> p s c", c=V)
                wsl = woy[:, fsl].rearrange("p (s c) -> p s c", c=V)
                gsl = gru_sb[:, fsl].rearrange("p (s c) -> p s c", c=V)
                psl = pred_sb[:, fsl].rearrange("p (s c) -> p s c", c=V)

                # tau = tanh(0.5*(zr_pre)); p1 = tau + 1  (=2*sigmoid(zr_pre))
                tzr = wpool.tile([128, SUBS * 56], F32, tag="tzr")
                tzr3 = tzr[:].rearrange("p (s c) -> p s c", c=56)
                nc.scalar.activation(tzr[:], A[:, :, 0:56], Act.Tanh, scale=0.5)
                nc.vector.tensor_scalar(tzr[:], tzr[:], 1.0, None, Alu.add)

                # hcand = tanh(xh + r*hm_h); r*hm_h = 0.5*p1_r*hm_h
                q2 = wpool.tile([128, SUBS * V], F32, tag="q2")
                q23 = q2[:].rearrange("p (s c) -> p s c", c=V)
                nc.vector.tensor_tensor(q23[:], tzr3[:, :, 28:56], Bv[:], Alu.mult)
                vv = wpool.tile([128, SUBS * V], F32, tag="vv")
                vv3 = vv[:].rearrange("p (s c) -> p s c", c=V)
                nc.vector.scalar_tensor_tensor(
                    vv3[:], q23[:], 0.5, A[:, :, 56:84], Alu.mult, Alu.add
                )
                hc = wpool.tile([128, SUBS * V], F32, tag="hc")
                hc3 = hc[:].rearrange("p (s c) -> p s c", c=V)
                nc.scalar.activation(hc[:], vv[:], Act.Tanh)

                # gru = hcand + z*(h-hcand);  z = 0.5*p1_z
                dd = wpool.tile([128, SUBS * V], F32, tag="dd")
                dd3 = dd[:].rearrange("p (s c) -> p s c", c=V)
                nc.vector.scalar_tensor_tensor(
                    dd3[:], hc3[:], -1.0, hsl[:], Alu.mult, Alu.add
                )
                qq = wpool.tile([128, SUBS * V], F32, tag="qq")
                qq3 = qq[:].rearrange("p (s c) -> p s c", c=V)
                nc.vector.tensor_tensor(qq3[:], tzr3[:, :, 0:28], dd3[:], Alu.mult)
                nc.vector.scalar_tensor_tensor(
                    gsl[:], qq3[:], 0.5, hc3[:], Alu.mult, Alu.add
                )

                # logits = rest + (woy' - 1); softmax over each 28-group
                t5 = wpool.tile([128, SUBS * V], F32, tag="t5")
                t53 = t5[:].rearrange("p (s c) -> p s c", c=V)
                nc.vector.scalar_tensor_tensor(
                    t53[:], wsl[:], -1.0, A[:, :, 84:112], Alu.add, Alu.add
                )
                ex = wpool.tile([128, SUBS * V], F32, tag="ex")
                ex3 = ex[:].rearrange("p (s c) -> p s c", c=V)
                nc.scalar.activation(ex[:], t5[:], Act.Exp)
                sm = wpool.tile([128, SUBS], F32, tag="sm")
                nc.vector.reduce_sum(sm[:], ex3[:], axis=mybir.AxisListType.X)
                rc = wpool.tile([128, SUBS], F32, tag="rc")
                nc.vector.reciprocal(rc[:], sm[:])
                for s in range(SUBS):
                    nc.vector.tensor_scalar(
                        psl[:, s, :], ex3[:, s, :], rc[:, s:s + 1], None, Alu.mult
                    )

                # stream finished quarters out
                if m % 4 == 3:
                    q = m // 4
                    osl = slice(q * (FLATW // 4), (q + 1) * (FLATW // 4))
                    nc.sync.dma_start(gru_o[:, osl], gru_sb[:, osl])
                    nc.sync.dma_start(pred_o[:, osl], pred_sb[:, osl])

    return nc


_NC_CACHE = None


def _get_nc():
    global _NC_CACHE
    if _NC_CACHE is None:
        _NC_CACHE = build_kernel()
    return _NC_CACHE


def kernel(inputs, prev_prediction, prev_state, gru_kernel, gru_rkernel,
           gru_bias, Wo, Uo, Co, Bo, emb):
    from concourse.bass_utils import run_bass_kernel_spmd

    inputs = np.asarray(inputs, np.float32)
    prev_prediction = np.asarray(prev_prediction)
    prev_state = np.asarray(prev_state, np.float32)
    gru_kernel = np.asarray(gru_kernel, np.float32)
    gru_rkernel = np.asarray(gru_rkernel, np.float32)
    gru_bias = np.asarray(gru_bias, np.float32)
    Wo_ = np.asarray(Wo, np.float32)
    Uo_ = np.asarray(Uo, np.float32)
    Co_ = np.asarray(Co, np.float32)
    Bo_ = np.asarray(Bo, np.float32)
    emb_ = np.asarray(emb, np.float32)

    # weight layout prep (pure concatenation / zero-padding, no arithmetic)
    WxA = np.concatenate([gru_kernel[:, 0:84], Co_], axis=1)          # [512,112]
    WhA = np.zeros((30, 112), np.float32)
    WhA[0:V, 0:56] = gru_rkernel[:, 0:56]
    WhA[0:V, 84:112] = Uo_
    WhA[28, 0:56] = gru_bias[0, 0:56]
    WhA[28, 56:84] = gru_bias[0, 56:84]
    WhA[28, 84:112] = Bo_[0]
    WhA[29, 0:56] = gru_bias[1, 0:56]
    WhB = np.zeros((30, V), np.float32)
    WhB[0:V, :] = gru_rkernel[:, 56:84]
    WhB[29, :] = gru_bias[1, 56:84]
    embT = np.ascontiguousarray(emb_.T)
    eye = np.eye(112, dtype=np.float32)

    nc = _get_nc()
    in_maps = []
    for c in range(NCORES):
        sl = slice(c * BC, (c + 1) * BC)
        xs = inputs[sl]
        hs = prev_state[sl]
        idx = prev_prediction[sl]
        hTv = np.empty((30, BC), np.float32)
        hTv[0:V] = hs.T
        hTv[28:30] = 1.0
        in_maps.append({
            "xT": np.ascontiguousarray(xs.T),
            "hT": hTv,
            "hflat": np.ascontiguousarray(
                hs.reshape(ROWG, 128, V).swapaxes(0, 1).reshape(128, FLATW)),
            "idxbf": np.ascontiguousarray(
                idx.astype(ml_dtypes.bfloat16)
                .reshape(ROWG, 128, V).swapaxes(0, 1).reshape(128, FLATW)),
            "WxA": WxA, "WhA": WhA, "WhB": WhB,
            "embT": embT, "Wo": Wo_, "eye": eye,
        })

    res = run_bass_kernel_spmd(nc, in_maps, core_ids=list(range(NCORES)))

    pred = np.empty((B, V), np.float32)
    gru = np.empty((B, V), np.float32)
    for c in range(NCORES):
        sl = slice(c * BC, (c + 1) * BC)
        pred[sl] = (res.results[c]["pred"].reshape(128, ROWG, V)
                    .swapaxes(0, 1).reshape(BC, V))
        gru[sl] = (res.results[c]["gru"].reshape(128, ROWG, V)
                   .swapaxes(0, 1).reshape(BC, V))
    return pred, gru


# revision 13
# speedup vs baseline: 8.8968x; 8.8968x over previous
"""Trainium2 Bass kernel for nn_CascadedGruCell.

Reference computation (per batch row b, F=512, V=28):
    xm   = x @ K + b0;  hm = h @ R + b1          (GRU, reset_after)
    z    = sigmoid(xm_z + hm_z)
    r    = sigmoid(xm_r + hm_r)
    hcand= tanh(xm_h + r * hm_h)
    gru  = z*h + (1-z)*hcand
    WoY[b,v] = (emb @ Wo)[idx[b,v]]              (28-entry table gather)
    pred = softmax(WoY + h @ Uo + x @ Co + Bo)

Strategy: pure data parallel over 8 cores (8192 rows each). Host does
zero-FLOP layout prep (transposes / concatenation / padding); device does
all arithmetic. Per core:
  - PE: out[m,n] = sum_f W[f,m] * xT[f,n]  (weights stationary, batch on
    the moving dim, N=512) accumulating x- and h-side into PSUM [112,512],
    then PE transpose-back of 128-batch blocks into row-per-partition
    layout for elementwise work.
  - sigmoid via tanh (0.5+0.5*tanh(x/2)) so ACT keeps one table set
    {tanh, exp} loaded.
  - table gather: 28x (is_equal mask * (T[k]+1) at bf16, max-accumulate).
"""

import sys

for _p in ("/opt/trn_rl_repo", "/root/.axon_site/_ro/trn_rl_repo"):
    if _p not in sys.path:
        sys.path.insert(0, _p)

import ml_dtypes
import numpy as np

import concourse.bass as bass
import concourse.mybir as mybir
from concourse.tile import TileContext

B, F, V = 65536, 512, 28
NCORES = 8
BC = B // NCORES            # 8192 rows per core
MACRO = 512                 # batch rows per matmul macro-tile
NMACRO = BC // MACRO        # 16
SUBS = MACRO // 128         # 4 transpose sub-blocks per macro
FLATW = BC * V // 128       # 1792 free elems of the [128, *] flat layout
ROWG = BC // 128            # 64 row-groups of 28 in the flat layout

F32 = mybir.dt.float32
BF16 = mybir.dt.bfloat16
Alu = mybir.AluOpType
Act = mybir.ActivationFunctionType


def _patch_tail_drain():
    """The walrus build in this container rejects >1-2 sync waits on one
    CTRL instruction; TileContext's tail drain attaches one wait per live
    sem lane. Split them across single-wait nops. Also cap the HWDGE DMA
    sem lanes at 2 so consumers carry fewer distinct waits."""
    import os
    import concourse.tile_sem_assignment as _tsa
    _tsa.NUM_HWDGE_SEMS = int(os.environ.get("K_DMA_LANES", "8"))
    from concourse.tile import TileContext as TC
    from bass_rust import ScopedClock, VectorClock

    if getattr(TC, "_drain_split_patched", False):
        return

    def _drain_and_barrier(self, tick_clock, wait_clock):
        gc = tick_clock.global_clock
        ticks = list(gc)
        n = len(ticks)
        seen = [0] * n
        for p in [i for i, t in enumerate(ticks) if t > 0]:
            vec = list(seen)
            vec[p] = ticks[p]
            nop = self.nc.sync.nop(nofuse=True, hint="tail_drain_split")
            wait_clock.add_sem_waits(
                nop.ins,
                ScopedClock({None: VectorClock(vec)}),
                ScopedClock({None: VectorClock(seen)}),
            )
            seen[p] = ticks[p]
        drain_inst = self.nc.sync.drain()
        wait_clock.add_sem_waits(
            drain_inst.ins,
            ScopedClock({None: gc}),
            ScopedClock({None: VectorClock(seen)}),
        )
        self.nc.all_engine_barrier()
        assert self.sems is not None
        popped = self.nc._tile_sem_poison_stack.pop()
        assert popped is self._sem_poison
        self.nc.clear_and_free_semaphores(list(self.sems.allocated().values()))
        self.nc.all_engine_barrier()

    TC._drain_and_barrier = _drain_and_barrier
    TC._drain_split_patched = True


def _split_excess_waits(nc, max_waits=1):
    """This container's walrus rejects instructions with more than ~1 sync
    wait. Hoist excess waits onto dedicated nops inserted immediately
    before the instruction on the same engine (per-engine program order
    makes sequential waits equivalent to one multi-wait)."""
    nid = [0]
    for fn in nc.m.functions:
        for bb in fn.blocks:
            out = []
            changed = False
            for ins in bb.instructions:
                si = ins.sync_info
                if si is not None and si.on_wait and len(si.on_wait) > max_waits:
                    waits = list(si.on_wait)
                    keep = waits[:max_waits]
                    for w in waits[max_waits:]:
                        nop = mybir.InstNoOp(
                            name=f"waitsplit_{nid[0]}", ins=[], outs=[]
                        )
                        nid[0] += 1
                        nop.engine = ins.engine
                        nop.sync_info = mybir.SyncInfo(
                            on_wait=[w], on_update=[]
                        )
                        out.append(nop)
                    ins.sync_info = mybir.SyncInfo(
                        on_wait=keep, on_update=list(si.on_update)
                    )
                    changed = True
                out.append(ins)
            if changed:
                bb.instructions = out


def build_kernel(reps=1):
    _patch_tail_drain()
    nc = bass.Bass()

    xT = nc.dram_tensor("xT", [F, BC], F32, kind="ExternalInput")
    hT = nc.dram_tensor("hT", [30, BC], F32, kind="ExternalInput")
    hflat = nc.dram_tensor("hflat", [128, FLATW], F32, kind="ExternalInput")
    idxbf = nc.dram_tensor("idxbf", [128, FLATW], BF16, kind="ExternalInput")
    WxA = nc.dram_tensor("WxA", [F, 112], F32, kind="ExternalInput")
    WhA = nc.dram_tensor("WhA", [30, 112], F32, kind="ExternalInput")
    WhB = nc.dram_tensor("WhB", [30, V], F32, kind="ExternalInput")
    embT = nc.dram_tensor("embT", [V, V], F32, kind="ExternalInput")
    Wo = nc.dram_tensor("Wo", [V, 1], F32, kind="ExternalInput")
    eye = nc.dram_tensor("eye", [112, 112], F32, kind="ExternalInput")

    pred_o = nc.dram_tensor("pred", [128, FLATW], F32, kind="ExternalOutput")
    gru_o = nc.dram_tensor("gru", [128, FLATW], F32, kind="ExternalOutput")

    with TileContext(nc) as tc:
        with (
            tc.tile_pool(name="const", bufs=1) as cpool,
            tc.tile_pool(name="flat", bufs=1) as fpool,
            tc.tile_pool(name="xtiles", bufs=3) as xpool,
            tc.tile_pool(name="work", bufs=3) as wpool,
            tc.tile_pool(name="psum", bufs=2, space="PSUM") as ppool,
        ):
            # ---- constants into SBUF ----
            wx_sb = cpool.tile([128, 4 * 112], F32, tag="wx")
            for g in range(4):
                nc.sync.dma_start(
                    wx_sb[:, g * 112:(g + 1) * 112],
                    WxA[g * 128:(g + 1) * 128, :],
                )
            wha_sb = cpool.tile([30, 112], F32, tag="wha")
            nc.sync.dma_start(wha_sb[:], WhA[:])
            whb_sb = cpool.tile([30, V], F32, tag="whb")
            nc.sync.dma_start(whb_sb[:], WhB[:])
            embT_sb = cpool.tile([V, V], F32, tag="embT")
            nc.sync.dma_start(embT_sb[:], embT[:])
            wo_sb = cpool.tile([V, 1], F32, tag="wo")
            nc.sync.dma_start(wo_sb[:], Wo[:])
            eye_sb = cpool.tile([112, 112], F32, tag="eye")
            nc.sync.dma_start(eye_sb[:], eye[:])

            for rep in range(reps):
                _emit_body(nc, tc, cpool, fpool, xpool, wpool, ppool, rep,
                           xT, hT, hflat, idxbf, pred_o, gru_o,
                           wx_sb, wha_sb, whb_sb, embT_sb, wo_sb, eye_sb)
    _split_excess_waits(nc)
    return nc


def _emit_body(nc, tc, cpool, fpool, xpool, wpool, ppool, rep,
               xT, hT, hflat, idxbf, pred_o, gru_o,
               wx_sb, wha_sb, whb_sb, embT_sb, wo_sb, eye_sb):
    if True:
        if True:
            hflat_sb = fpool.tile([128, FLATW], F32, tag="hflat")
            nc.sync.dma_start(hflat_sb[:], hflat[:])
            idx_sb = fpool.tile([128, FLATW], BF16, tag="idx")
            nc.sync.dma_start(idx_sb[:], idxbf[:])

            gru_sb = fpool.tile([128, FLATW], F32, tag="gru_out")
            pred_sb = fpool.tile([128, FLATW], F32, tag="pred_out")

            # ---- table = emb @ Wo, broadcast to all partitions, +1 ----
            ones_sb = cpool.tile([1, 128], F32, tag="ones")
            nc.vector.memset(ones_sb[:], 1.0)
            ps_t = ppool.tile([1, V], F32, tag="psX", name=f"ps_t_{rep}")
            nc.tensor.matmul(ps_t[:], wo_sb[:], embT_sb[:], start=True, stop=True)
            tbl1 = cpool.tile([1, V], F32, tag="tbl1")
            nc.scalar.copy(tbl1[:], ps_t[:])
            ps_b = ppool.tile([128, V], F32, tag="psH", name=f"ps_b_{rep}")
            nc.tensor.matmul(ps_b[:], ones_sb[:], tbl1[:], start=True, stop=True)
            tblB = cpool.tile([128, V], F32, tag="tblB")
            nc.vector.tensor_scalar(tblB[:], ps_b[:], 0.0, None, Alu.add)

            # ---- WoY gather: woy = sum_k (idx==k) * T[k]  (bf16, disjoint
            # masks). Two parallel accumulation chains: DVE and GPSIMD. ----
            import os
            NGP = int(os.environ.get("K_NGP", "0"))
            woy = fpool.tile([128, FLATW], BF16, tag="woy")
            nc.vector.memset(woy[:], 0.0)
            woyB = fpool.tile([128, FLATW], BF16, tag="woyB")
            nc.gpsimd.memset(woyB[:], 0.0)
            for k in range(V):
                gtmp = wpool.tile([128, FLATW], BF16, tag="gtmp",
                                  name=f"gtmp_{rep}_{k}")
                nc.vector.tensor_scalar(
                    gtmp[:], idx_sb[:], float(k), tblB[:, k:k + 1],
                    Alu.is_equal, Alu.mult,
                )
                if k < V - NGP:
                    nc.vector.tensor_tensor(woy[:], woy[:], gtmp[:], Alu.add)
                else:
                    nc.gpsimd.tensor_tensor(woyB[:], woyB[:], gtmp[:], Alu.add)
            nc.vector.tensor_tensor(woy[:], woy[:], woyB[:], Alu.add)

            # ---- main loop over macro-tiles ----
            QW = 4 * MACRO  # batch columns per quarter (2048)
            xTg = xT[:].rearrange("(g p) n -> p g n", g=4)
            for m in range(NMACRO):
                n0 = m * MACRO
                q, mm = divmod(m, 4)
                if mm == 0:
                    xbig = xpool.tile([128, 4 * QW], F32, tag="xbig",
                                      name=f"xbig_{rep}_{q}")
                    nc.sync.dma_start(
                        xbig[:].rearrange("p (g n) -> p g n", g=4),
                        xTg[:, :, q * QW:(q + 1) * QW],
                    )
                    htq = xpool.tile([30, QW], F32, tag="htq", name=f"htq_{rep}_{q}")
                    nc.scalar.dma_start(htq[:], hT[:, q * QW:(q + 1) * QW])
                xt = [
                    xbig[:, g * QW + mm * MACRO: g * QW + (mm + 1) * MACRO]
                    for g in range(4)
                ]
                ht = htq[:, mm * MACRO:(mm + 1) * MACRO]

                psX = ppool.tile([112, MACRO], F32, tag="psX")
                psH = ppool.tile([V, MACRO], F32, tag="psH")
                for g in range(4):
                    nc.tensor.matmul(
                        psX[:], wx_sb[:, g * 112:(g + 1) * 112], xt[g],
                        start=(g == 0), stop=False,
                    )
                nc.tensor.matmul(psX[:], wha_sb[:], ht, start=False, stop=True)
                nc.tensor.matmul(psH[:], whb_sb[:], ht, start=True, stop=True)

                # PSUM -> SBUF (one wide copy each), then PE transpose-back
                sbX = wpool.tile([112, MACRO], F32, tag="sbX")
                nc.scalar.copy(sbX[:], psX[:])
                sbH = wpool.tile([V, MACRO], F32, tag="sbH")
                nc.scalar.copy(sbH[:], psH[:])

                ptA = ppool.tile([128, SUBS * 112], F32, tag="ptA")
                ptB = ppool.tile([128, SUBS * V], F32, tag="ptB")
                for s in range(SUBS):
                    nc.tensor.transpose(
                        ptA[:, s * 112:(s + 1) * 112],
                        sbX[:, s * 128:(s + 1) * 128],
                        eye_sb[:],
                    )
                    nc.tensor.transpose(
                        ptB[:, s * V:(s + 1) * V],
                        sbH[:, s * 128:(s + 1) * 128],
                        eye_sb[0:V, 0:V],
                    )

                # row-per-partition views: ptA blocks [s] = [zr(56) xh(28) rest(28)]
                A = ptA[:].rearrange("p (s c) -> p s c", c=112)
                Bv = ptB[:].rearrange("p (s c) -> p s c", c=V)
                fsl = slice(SUBS * V * m, SUBS * V * (m + 1))
                hsl = hflat_sb[:, fsl].rearrange("p (s c) -> p s c", c=V)
                wsl = woy[:, fsl].rearrange("p (s c) -> p s c", c=V)
                gsl = gru_sb[:, fsl].rearrange("p (s c) -> p s c", c=V)
                psl = pred_sb[:, fsl].rearrange("p (s c) -> p s c", c=V)

                # tau = tanh(0.5*(zr_pre)); p1 = tau + 1  (=2*sigmoid(zr_pre))
                tzr = wpool.tile([128, SUBS * 56], F32, tag="tzr")
                tzr3 = tzr[:].rearrange("p (s c) -> p s c", c=56)
                nc.scalar.activation(tzr[:], A[:, :, 0:56], Act.Tanh, scale=0.5)
                nc.vector.tensor_scalar(tzr[:], tzr[:], 1.0, None, Alu.add)

                # hcand = tanh(xh + r*hm_h); r*hm_h = 0.5*p1_r*hm_h
                q2 = wpool.tile([128, SUBS * V], F32, tag="q2")
                q23 = q2[:].rearrange("p (s c) -> p s c", c=V)
                nc.vector.tensor_tensor(q23[:], tzr3[:, :, 28:56], Bv[:], Alu.mult)
                vv = wpool.tile([128, SUBS * V], F32, tag="vv")
                vv3 = vv[:].rearrange("p (s c) -> p s c", c=V)
                nc.vector.scalar_tensor_tensor(
                    vv3[:], q23[:], 0.5, A[:, :, 56:84], Alu.mult, Alu.add
                )
                hc = wpool.tile([128, SUBS * V], F32, tag="hc")
                hc3 = hc[:].rearrange("p (s c) -> p s c", c=V)
                nc.scalar.activation(hc[:], vv[:], Act.Tanh)

                # gru = hcand + z*(h-hcand);  z = 0.5*p1_z
                dd = wpool.tile([128, SUBS * V], F32, tag="dd")
                dd3 = dd[:].rearrange("p (s c) -> p s c", c=V)
                nc.vector.scalar_tensor_tensor(
                    dd3[:], hc3[:], -1.0, hsl[:], Alu.mult, Alu.add
                )
                qq = wpool.tile([128, SUBS * V], F32, tag="qq")
                qq3 = qq[:].rearrange("p (s c) -> p s c", c=V)
                nc.vector.tensor_tensor(qq3[:], tzr3[:, :, 0:28], dd3[:], Alu.mult)
                nc.vector.scalar_tensor_tensor(
                    gsl[:], qq3[:], 0.5, hc3[:], Alu.mult, Alu.add
                )

                # logits = rest + woy; softmax over each 28-group
                t5 = wpool.tile([128, SUBS * V], F32, tag="t5")
                t53 = t5[:].rearrange("p (s c) -> p s c", c=V)
                eng_t5 = nc.gpsimd if os.environ.get("K_GPELEM", "0") == "1" else nc.vector
                eng_t5.tensor_tensor(t53[:], wsl[:], A[:, :, 84:112], Alu.add)
                ex = wpool.tile([128, SUBS * V], F32, tag="ex")
                ex3 = ex[:].rearrange("p (s c) -> p s c", c=V)
                nc.scalar.activation(ex[:], t5[:], Act.Exp)
                sm = wpool.tile([128, SUBS], F32, tag="sm")
                nc.vector.reduce_sum(sm[:], ex3[:], axis=mybir.AxisListType.X)
                rc = wpool.tile([128, SUBS], F32, tag="rc")
                nc.vector.reciprocal(rc[:], sm[:])
                eng_pr = nc.gpsimd if os.environ.get("K_GPELEM", "0") == "1" else nc.vector
                for s in range(SUBS):
                    eng_pr.tensor_scalar(
                        psl[:, s, :], ex3[:, s, :], rc[:, s:s + 1], None, Alu.mult
                    )

                # stream finished quarters out
                if m % 4 == 3:
                    q = m // 4
                    osl = slice(q * (FLATW // 4), (q + 1) * (FLATW // 4))
                    nc.scalar.dma_start(gru_o[:, osl], gru_sb[:, osl])
                    nc.sync.dma_start(pred_o[:, osl], pred_sb[:, osl])


_NC_CACHE = None


def _get_nc():
    global _NC_CACHE
    if _NC_CACHE is None:
        _NC_CACHE = build_kernel()
    return _NC_CACHE


def kernel(inputs, prev_prediction, prev_state, gru_kernel, gru_rkernel,
           gru_bias, Wo, Uo, Co, Bo, emb):
    from concourse.bass_utils import run_bass_kernel_spmd

    inputs = np.asarray(inputs, np.float32)
    prev_prediction = np.asarray(prev_prediction)
    prev_state = np.asarray(prev_state, np.float32)
    gru_kernel = np.asarray(gru_kernel, np.float32)
    gru_rkernel = np.asarray(gru_rkernel, np.float32)
    gru_bias = np.asarray(gru_bias, np.float32)
    Wo_ = np.asarray(Wo, np.float32)
    Uo_ = np.asarray(Uo, np.float32)
    Co_ = np.asarray(Co, np.float32)
    Bo_ = np.asarray(Bo, np.float32)
    emb_ = np.asarray(emb, np.float32)

    # weight layout prep (pure concatenation / zero-padding, no arithmetic)
    WxA = np.concatenate([gru_kernel[:, 0:84], Co_], axis=1)          # [512,112]
    WhA = np.zeros((30, 112), np.float32)
    WhA[0:V, 0:56] = gru_rkernel[:, 0:56]
    WhA[0:V, 84:112] = Uo_
    WhA[28, 0:56] = gru_bias[0, 0:56]
    WhA[28, 56:84] = gru_bias[0, 56:84]
    WhA[28, 84:112] = Bo_[0]
    WhA[29, 0:56] = gru_bias[1, 0:56]
    WhB = np.zeros((30, V), np.float32)
    WhB[0:V, :] = gru_rkernel[:, 56:84]
    WhB[29, :] = gru_bias[1, 56:84]
    embT = np.ascontiguousarray(emb_.T)
    eye = np.eye(112, dtype=np.float32)

    nc = _get_nc()
    in_maps = []
    for c in range(NCORES):
        sl = slice(c * BC, (c + 1) * BC)
        xs = inputs[sl]
        hs = prev_state[sl]
        idx = prev_prediction[sl]
        hTv = np.empty((30, BC), np.float32)
        hTv[0:V] = hs.T
        hTv[28:30] = 1.0
        in_maps.append({
            "xT": np.ascontiguousarray(xs.T),
            "hT": hTv,
            "hflat": np.ascontiguousarray(
                hs.reshape(ROWG, 128, V).swapaxes(0, 1).reshape(128, FLATW)),
            "idxbf": np.ascontiguousarray(
                idx.astype(ml_dtypes.bfloat16)
                .reshape(ROWG, 128, V).swapaxes(0, 1).reshape(128, FLATW)),
            "WxA": WxA, "WhA": WhA, "WhB": WhB,
            "embT": embT, "Wo": Wo_, "eye": eye,
        })

    res = run_bass_kernel_spmd(nc, in_maps, core_ids=list(range(NCORES)))

    pred = np.empty((B, V), np.float32)
    gru = np.empty((B, V), np.float32)
    for c in range(NCORES):
        sl = slice(c * BC, (c + 1) * BC)
        pred[sl] = (res.results[c]["pred"].reshape(128, ROWG, V)
                    .swapaxes(0, 1).reshape(BC, V))
        gru[sl] = (res.results[c]["gru"].reshape(128, ROWG, V)
                   .swapaxes(0, 1).reshape(BC, V))
    return pred, gru
